# revision 10
# baseline (speedup 1.0000x reference)
"""BailingMoE block on 8 Trainium2 NeuronCores — v2 (fp8 expert path).

Sharding (unchanged from v1):
  - Attention: data-parallel over tokens (core i owns tokens [128i, 128(i+1))).
    k heads and v are AllGathered (bf16) in two separate collectives so score
    compute can start while v is still in flight.
  - Router: fp32 per-chunk, bit-identical to v1 (one borderline token makes
    the top-2 decision a knife's edge: attention+router numerics are FROZEN).
  - MoE routed experts: expert-parallel, dense over all 1024 tokens, weighted
    per-token (0 when unrouted). h2 ships as fp8e4m3 (halves the AllGather),
    expert matmuls run fp8 DoubleRow (2 k-tiles/instr, 0.5 cyc/row). w_gu/w_d
    are host-scaled x16 to clear the fp8 subnormal floor; the 1/16s fold into
    the silu scale and the esel gating column.
  - Shared expert: bf16 on own chunk, computed during the h2 AllGather.
  - One ReduceScatter (bf16) combines routed partials across cores.
"""

import numpy as np

import concourse.bass as bass
import concourse.bacc as bacc
import concourse.mybir as mybir
import concourse.tile as tile
from concourse.bass_utils import run_bass_kernel_spmd
from concourse.masks import make_identity

F32 = mybir.dt.float32
BF16 = mybir.dt.bfloat16
F8 = mybir.dt.float8e4
AF = mybir.ActivationFunctionType
ALU = mybir.AluOpType
AX = mybir.AxisListType
DR = mybir.MatmulPerfMode.DoubleRow

N_CORES = 8
T = 1024          # tokens
TC = 128          # tokens per core chunk
H = 2048          # hidden
NH = 16           # q heads
NKV = 4           # kv heads
DH = 128          # head dim
E = 8             # experts
I = 1024          # moe intermediate
IS = 1024         # shared intermediate
QKV = (NH + 2 * NKV) * DH  # 3072
KH = H // 128     # 16 k-tiles over hidden
EPS = 1e-6
SCALE = DH ** -0.5
NEG = -1e9
WSCL = 16.0       # fp8 weight pre-scale (host); /16 folded into silu + esel

_cache = {}


def _bc(ap, n, axis=1):
    """Insert a broadcast (step 0, count n) free dim into an AP at `axis`."""
    a = [list(p) for p in ap.ap]
    a.insert(axis, [0, n])
    return bass.AP(tensor=ap.tensor, offset=ap.offset, ap=a)


def build_nc():
    nc = bacc.Bacc("TRN2", target_bir_lowering=False, num_devices=N_CORES)

    # ---- I/O ----
    x_chunk = nc.dram_tensor("x_chunk", [TC, H], F32, kind="ExternalInput")
    wqkv_l = nc.dram_tensor("wqkv_l", [6, 128, 16 * 512], BF16, kind="ExternalInput")
    wo_l = nc.dram_tensor("wo_l", [4, 128, 16 * 512], BF16, kind="ExternalInput")
    wgu8_l = nc.dram_tensor("wgu8_l", [128, 16, 16, 2, 128], F8, kind="ExternalInput")
    wd8_l = nc.dram_tensor("wd8_l", [128, 4, 2, 4, 512], F8, kind="ExternalInput")
    wsgu_l = nc.dram_tensor("wsgu_l", [4, 128, 16 * 512], BF16, kind="ExternalInput")
    wsd_l = nc.dram_tensor("wsd_l", [4, 128, 8 * 512], BF16, kind="ExternalInput")
    wrT = nc.dram_tensor("wrT", [H, E], F32, kind="ExternalInput")
    rope_q = nc.dram_tensor("rope_q", [TC, 4, DH // 2], F32, kind="ExternalInput")
    rope_k = nc.dram_tensor("rope_k", [TC, 4, DH // 2], F32, kind="ExternalInput")
    mask_in = nc.dram_tensor("mask_in", [T, TC], F32, kind="ExternalInput")
    esel = nc.dram_tensor("esel", [1, E], F32, kind="ExternalInput")
    out_chunk = nc.dram_tensor("out_chunk", [TC, H], F32, kind="ExternalOutput")

    rg = [list(range(N_CORES))]

    with tile.TileContext(nc) as tc:
        with tc.tile_pool(name="dram", bufs=1, space="DRAM") as dram, \
             tc.tile_pool(name="const", bufs=1) as const, \
             tc.tile_pool(name="mid", bufs=1) as mid, \
             tc.tile_pool(name="sb", bufs=2) as sb:

            # ---- DRAM collective buffers ----
            KSZ = NKV * DH * TC
            k_in = dram.tile([KSZ], BF16)
            k_out = dram.tile([N_CORES * KSZ], BF16, addr_space="Shared")
            v_in = dram.tile([TC, NKV * DH], BF16)
            v_out = dram.tile([N_CORES * TC, NKV * DH], BF16, addr_space="Shared")
            h8_in = dram.tile([H, TC], F8)
            h8_out = dram.tile([N_CORES * H, TC], F8, addr_space="Shared")
            w_in = dram.tile([TC, E], F32)
            w_out = dram.tile([T, E], F32, addr_space="Shared")
            rs_in = dram.tile([T, H], BF16)
            rs_out = dram.tile([TC, H], BF16)

            # ---- constants ----
            ident_bf = const.tile([128, 128], BF16)
            make_identity(nc, ident_bf)
            ident_f = const.tile([128, 128], F32)
            make_identity(nc, ident_f)
            eps_sb = const.tile([128, 1], F32)
            nc.vector.memset(eps_sb, EPS)
            esel_sb = const.tile([128, E], F32)
            nc.sync.dma_start(
                out=esel_sb,
                in_=bass.AP(tensor=esel, offset=0, ap=[[0, 128], [1, E]]))
            mask_sb = const.tile([128, N_CORES, TC], F32)
            nc.sync.dma_start(
                out=mask_sb,
                in_=bass.AP(tensor=mask_in, offset=0,
                            ap=[[TC, 128], [128 * TC, N_CORES], [1, TC]]))
            wrT_sb = const.tile([128, KH, E], F32)
            nc.sync.dma_start(
                out=wrT_sb,
                in_=bass.AP(tensor=wrT, offset=0,
                            ap=[[E, 128], [128 * E, KH], [1, E]]))

            # ---- persistent (cross-phase) tiles ----
            x2_sb = mid.tile([TC, H], F32)
            shared_sb = mid.tile([TC, H], F32)
            h2_f = mid.tile([TC, H], F32)
            wcols = mid.tile([TC, N_CORES], F32)  # own-expert weight per token
            h2_bf = mid.tile([TC, H], BF16)
            h2Ts = [mid.tile([128, TC], BF16, tag=f"h2Ts{j}", name=f"h2Ts{j}")
                    for j in range(KH)]

            def rms_scale(xt, d, tag):
                sq = sb.tile([TC, H], F32, tag="rmssq", bufs=1)
                nc.vector.tensor_mul(sq[:, :d], xt, xt)
                red = sb.tile([TC, 1], F32, tag=f"rred{tag}")
                nc.vector.tensor_reduce(red, sq[:, :d], axis=AX.X, op=ALU.add)
                nc.scalar.activation(red, red, AF.Sqrt, bias=eps_sb[:TC], scale=1.0 / d)
                nc.vector.reciprocal(red, red)
                return red

            # ================= ATTENTION PHASE (numerics frozen vs v1) ======
            with tc.tile_pool(name="apool", bufs=2) as ap_, \
                 tc.tile_pool(name="wstream", bufs=2) as wstream, \
                 tc.tile_pool(name="ps_big", bufs=3, space="PSUM") as ps_big, \
                 tc.tile_pool(name="ps_sm", bufs=2, space="PSUM") as ps_sm:
                x_sb = ap_.tile([TC, H], F32, tag="x_sb", bufs=1)
                nc.sync.dma_start(out=x_sb, in_=x_chunk[:, :])
                rope_q_sb = ap_.tile([TC, 4, DH // 2], F32, tag="ropeq", bufs=1)
                nc.sync.dma_start(out=rope_q_sb, in_=rope_q[:, :, :])
                rope_k_sb = ap_.tile([TC, 4, DH // 2], F32, tag="ropek", bufs=1)
                nc.sync.dma_start(out=rope_k_sb, in_=rope_k[:, :, :])

                rs1 = rms_scale(x_sb, H, "1")
                qpool_cm = tc.tile_pool(name="qpool", bufs=1)
                qp_ = qpool_cm.__enter__()
                h1_bf = qp_.tile([TC, H], BF16, tag="h1bf", bufs=1)
                nc.vector.tensor_scalar_mul(h1_bf, x_sb, rs1)

                # h1T via PE transpose
                h1T = []
                for j in range(KH):
                    pt = ps_sm.tile([128, 128], BF16, tag="pstb")
                    nc.tensor.transpose(pt, h1_bf[:, j * 128:(j + 1) * 128], ident_bf)
                    t_ = qp_.tile([128, TC], BF16, tag=f"h1T{j}", bufs=1)
                    nc.vector.tensor_copy(t_, pt)
                    h1T.append(t_)

                # qkv = h1 @ wqkv -> [TC, 3072] fp32 ; k/v chunks first
                qkv_f = qp_.tile([TC, QKV], F32, tag="qkvf", bufs=1)

                def qkv_chunk(n):
                    wk = wstream.tile([128, KH, 512], BF16, tag="wst")
                    (nc.sync if n % 2 == 0 else nc.scalar).dma_start(
                        out=wk,
                        in_=bass.AP(tensor=wqkv_l, offset=n * 128 * KH * 512,
                                    ap=[[KH * 512, 128], [1, KH * 512]]))
                    pq = ps_big.tile([TC, 512], F32, tag="mm512")
                    for k in range(KH):
                        nc.tensor.matmul(pq, h1T[k], wk[:, k, :],
                                         start=(k == 0), stop=(k == KH - 1))
                    nc.vector.tensor_copy(qkv_f[:, n * 512:(n + 1) * 512], pq)

                for n in (4, 5):
                    qkv_chunk(n)

                q3 = qkv_f[:, 0:NH * DH].rearrange("p (h d) -> p h d", h=NH)
                k3 = qkv_f[:, NH * DH:(NH + NKV) * DH].rearrange(
                    "p (h d) -> p h d", h=NKV)
                v2d = qkv_f[:, (NH + NKV) * DH:]

                def qk_norm(x3, nh, tag):
                    sq = sb.tile([TC, H], F32, tag="rmssq", bufs=1)
                    x2dv = x3.rearrange("p h d -> p (h d)")
                    nc.vector.tensor_mul(sq[:, :nh * DH], x2dv, x2dv)
                    red = qp_.tile([TC, nh, 1], F32, tag=f"qred{tag}")
                    nc.vector.tensor_reduce(
                        red, sq[:, :nh * DH].rearrange("p (h d) -> p h d", h=nh),
                        axis=AX.X, op=ALU.add)
                    nc.scalar.activation(
                        red.rearrange("p h one -> p (h one)"),
                        red.rearrange("p h one -> p (h one)"),
                        AF.Sqrt, bias=eps_sb[:TC], scale=1.0 / DH)
                    nc.vector.reciprocal(
                        red.rearrange("p h one -> p (h one)"),
                        red.rearrange("p h one -> p (h one)"))
                    for h in range(nh):
                        nc.vector.tensor_scalar_mul(
                            x3[:, h, :], x3[:, h, :], red[:, h, :])

                qk_norm(k3, NKV, "k")

                qkv_bf = qp_.tile([TC, QKV], BF16, tag="qkvbf", bufs=1)
                qbf3 = qkv_bf[:, 0:NH * DH].rearrange("p (h d) -> p h d", h=NH)
                kbf3 = qkv_bf[:, NH * DH:(NH + NKV) * DH].rearrange(
                    "p (h d) -> p h d", h=NKV)

                def rope(x3, obf3, nh, tab):
                    c1 = _bc(tab[:, 0, :], nh)
                    s1 = _bc(tab[:, 1, :], nh)
                    c2 = _bc(tab[:, 2, :], nh)
                    s2 = _bc(tab[:, 3, :], nh)
                    x1 = x3[:, :, 0:DH // 2]
                    x2 = x3[:, :, DH // 2:DH]
                    t1 = qp_.tile([TC, NH, DH // 2], F32, tag="rp1", bufs=1)
                    tn = qp_.tile([TC, NH, DH // 2], F32, tag="rpn", bufs=1)
                    t1v = t1[:, :nh, :]
                    tnv = tn[:, :nh, :]
                    nc.vector.tensor_mul(t1v, x1, c1)
                    nc.vector.tensor_mul(tnv, x2, s1)
                    nc.vector.tensor_sub(t1v, t1v, tnv)
                    nc.vector.tensor_copy(obf3[:, :, 0:DH // 2], t1v)
                    nc.vector.tensor_mul(t1v, x2, c2)
                    nc.vector.tensor_mul(tnv, x1, s2)
                    nc.vector.tensor_add(t1v, t1v, tnv)
                    nc.vector.tensor_copy(obf3[:, :, DH // 2:DH], t1v)

                rope(k3, kbf3, NKV, rope_k_sb)
                nc.vector.tensor_copy(qkv_bf[:, (NH + NKV) * DH:], v2d)

                # kT -> one staging tile -> DRAM -> AllGather (k first)
                kT_stage = qp_.tile([DH, NKV, TC], BF16, tag="kTst", bufs=1)
                for g in range(NKV):
                    pt = ps_sm.tile([128, 128], BF16, tag="pstb")
                    nc.tensor.transpose(
                        pt, qkv_bf[:, (NH + g) * DH:(NH + g + 1) * DH], ident_bf)
                    nc.vector.tensor_copy(kT_stage[:, g, :], pt)
                nc.sync.dma_start(
                    out=bass.AP(tensor=k_in.tensor, offset=k_in.offset,
                                ap=[[TC, DH], [DH * TC, NKV], [1, TC]]),
                    in_=kT_stage)
                nc.gpsimd.collective_compute(
                    "AllGather", ALU.bypass, replica_groups=rg,
                    ins=[k_in.opt()], outs=[k_out.opt()])

                # v rows -> DRAM -> AllGather (second)
                nc.sync.dma_start(
                    out=bass.AP(tensor=v_in.tensor, offset=v_in.offset,
                                ap=[[NKV * DH, TC], [1, NKV * DH]]),
                    in_=qkv_bf[:, (NH + NKV) * DH:])
                nc.gpsimd.collective_compute(
                    "AllGather", ALU.bypass, replica_groups=rg,
                    ins=[v_in.opt()], outs=[v_out.opt()])

                # q columns of the projection (overlaps the AllGathers)
                for n in range(4):
                    qkv_chunk(n)
                qk_norm(q3, NH, "q")
                rope(q3, qbf3, NH, rope_q_sb)
                qT = []
                for h in range(NH):
                    pt = ps_sm.tile([128, 128], BF16, tag="pstb")
                    nc.tensor.transpose(
                        pt, qkv_bf[:, h * DH:(h + 1) * DH], ident_bf)
                    t_ = ap_.tile([DH, TC], BF16, tag=f"qT{h}", bufs=1)
                    nc.vector.tensor_copy(t_, pt)
                    qT.append(t_)

                # prefetch wo chunks during AG window (2 dedicated + 2
                # through the weight stream once qkv frees it)
                wo_sb = []
                for n in range(4):
                    if n < 1:
                        wos = wstream.tile([128, KH, 512], BF16, tag=f"wo{n}",
                                           bufs=1, name=f"wo{n}")
                    else:
                        wos = wstream.tile([128, KH, 512], BF16, tag="wst")
                    (nc.sync if n % 2 == 0 else nc.scalar).dma_start(
                        out=wos,
                        in_=bass.AP(tensor=wo_l, offset=n * 128 * KH * 512,
                                    ap=[[KH * 512, 128], [1, KH * 512]]))
                    wo_sb.append(wos)

                qpool_cm.__exit__(None, None, None)
                pbfp_cm = tc.tile_pool(name="pbfp", bufs=1)
                pbfp = pbfp_cm.__enter__()

                # prefetch all kv-group tiles (v completes when its AG does)
                kT_gs, v_gs = [], []
                for g in range(NKV):
                    kT_g = ap_.tile([DH, N_CORES, TC], BF16, tag=f"kTg{g}",
                                    bufs=1, name=f"kTg{g}")
                    nc.scalar.dma_start(
                        out=kT_g,
                        in_=bass.AP(
                            tensor=k_out.tensor,
                            offset=k_out.offset + g * DH * TC,
                            ap=[[TC, DH], [KSZ, N_CORES], [1, TC]]))
                    kT_gs.append(kT_g)
                    v_g = ap_.tile([TC, N_CORES, DH + 1], BF16, tag=f"vg{g}",
                                   bufs=1, name=f"vg{g}")
                    nc.sync.dma_start(
                        out=v_g[:, :, 0:DH],
                        in_=bass.AP(
                            tensor=v_out.tensor,
                            offset=v_out.offset + g * DH,
                            ap=[[NKV * DH, TC], [TC * NKV * DH, N_CORES],
                                [1, DH]]))
                    nc.vector.memset(v_g[:, :, DH:DH + 1], 1.0)
                    v_gs.append(v_g)

                # all scores+exp first (overlaps the v AllGather), ctx after
                pbf = []
                for h in range(NH):
                    g = h // (NH // NKV)
                    probs = ap_.tile([128, N_CORES, TC], F32, tag="probs", bufs=2)
                    for half in range(2):
                        ps = ps_big.tile([TC, 512], F32, tag="mm512")
                        for jj in range(4):
                            j = half * 4 + jj
                            nc.tensor.matmul(
                                ps[:, jj * TC:(jj + 1) * TC],
                                kT_gs[g][:, j, :], qT[h], start=True, stop=True)
                        nc.vector.tensor_add(
                            probs.rearrange("p j q -> p (j q)")
                            [:, half * 512:(half + 1) * 512],
                            ps,
                            mask_sb.rearrange("p j q -> p (j q)")
                            [:, half * 512:(half + 1) * 512])
                    pflat = probs.rearrange("p j q -> p (j q)")
                    probs_bf = pbfp.tile([128, N_CORES, TC], BF16,
                                         tag=f"pbf{h}", bufs=1, name=f"pbf{h}")
                    nc.scalar.activation(
                        probs_bf.rearrange("p j q -> p (j q)"), pflat,
                        AF.Exp, scale=SCALE)
                    pbf.append(probs_bf)

                ctxT = []
                for h in range(NH):
                    g = h // (NH // NKV)
                    pctx = ps_sm.tile([TC, DH + 1], F32, tag="pctx", bufs=2)
                    for j in range(N_CORES):
                        nc.tensor.matmul(pctx, pbf[h][:, j, :],
                                         v_gs[g][:, j, :],
                                         start=(j == 0), stop=(j == N_CORES - 1))
                    rden = sb.tile([TC, 1], F32, tag="rden")
                    nc.vector.reciprocal(rden, pctx[:, DH:DH + 1])
                    ctx_bf = sb.tile([TC, DH], BF16, tag="ctxbf")
                    nc.vector.tensor_scalar_mul(ctx_bf, pctx[:, 0:DH], rden)
                    pt2 = ps_sm.tile([128, 128], BF16, tag="pstb")
                    nc.tensor.transpose(pt2, ctx_bf, ident_bf)
                    t_ = ap_.tile([DH, TC], BF16, tag=f"ctxT{h}", bufs=1)
                    nc.vector.tensor_copy(t_, pt2)
                    ctxT.append(t_)

                pbfp_cm.__exit__(None, None, None)

                # attn_out = ctx @ wo ; x2 = x + attn_out
                for n in range(H // 512):
                    po = ps_big.tile([TC, 512], F32, tag="mm512")
                    for k in range(KH):
                        nc.tensor.matmul(po, ctxT[k], wo_sb[n][:, k, :],
                                         start=(k == 0), stop=(k == KH - 1))
                    nc.vector.tensor_add(x2_sb[:, n * 512:(n + 1) * 512], po,
                                         x_sb[:, n * 512:(n + 1) * 512])

                # ---- h2 + router (frozen) ----
                rs2 = rms_scale(x2_sb, H, "2")
                nc.vector.tensor_scalar_mul(h2_f, x2_sb, rs2)

                # fp32 router on own chunk (identical to v1)
                pr = ps_big.tile([TC, E], F32, tag="mm512")
                for j in range(KH):
                    pt = ps_sm.tile([128, 128], F32, tag="pstf", bufs=1)
                    nc.tensor.transpose(pt, h2_f[:, j * 128:(j + 1) * 128], ident_f)
                    t_ = ap_.tile([128, TC], F32, tag="h2T32")
                    nc.vector.tensor_copy(t_, pt)
                    nc.tensor.matmul(pr, t_, wrT_sb[:, j, :],
                                     start=(j == 0), stop=(j == KH - 1))
                probs8 = sb.tile([TC, E], F32, tag="probs8")
                nc.scalar.activation(probs8, pr, AF.Exp, scale=1.0)
                den8 = sb.tile([TC, 1], F32, tag="den8")
                nc.vector.tensor_reduce(den8, probs8, axis=AX.X, op=ALU.add)
                rden8 = sb.tile([TC, 1], F32, tag="rden8")
                nc.vector.reciprocal(rden8, den8)
                nc.vector.tensor_scalar_mul(probs8, probs8, rden8)
                mx8 = sb.tile([TC, 8], F32, tag="mx8")
                nc.vector.max(out=mx8, in_=probs8)
                s12 = sb.tile([TC, 1], F32, tag="s12")
                nc.vector.tensor_add(s12, mx8[:, 0:1], mx8[:, 1:2])
                rs12 = sb.tile([TC, 1], F32, tag="rs12")
                nc.vector.reciprocal(rs12, s12)
                eq1 = sb.tile([TC, E], F32, tag="eq1")
                nc.vector.tensor_scalar(eq1, probs8, mx8[:, 0:1], None,
                                        op0=ALU.is_equal)
                eq2 = sb.tile([TC, E], F32, tag="eq2")
                nc.vector.tensor_scalar(eq2, probs8, mx8[:, 1:2], None,
                                        op0=ALU.is_equal)
                nc.vector.tensor_add(eq1, eq1, eq2)
                wm = sb.tile([TC, E], F32, tag="wm")
                nc.vector.tensor_mul(wm, probs8, eq1)
                nc.vector.tensor_scalar_mul(wm, wm, rs12)
                nc.sync.dma_start(out=w_in[:, :], in_=wm)

                # h2 -> bf16 transposes (reused by shared expert) -> fp8 ship
                nc.vector.tensor_copy(h2_bf, h2_f)
                h8_stage = ap_.tile([128, KH, 128], F8, tag="h8st", bufs=1)
                for j in range(KH):
                    pt = ps_sm.tile([128, 128], BF16, tag="pstb")
                    nc.tensor.transpose(pt, h2_bf[:, j * 128:(j + 1) * 128],
                                        ident_bf)
                    nc.vector.tensor_copy(h2Ts[j], pt)
                    nc.vector.tensor_copy(h8_stage[:, j, :], h2Ts[j])
                nc.sync.dma_start(
                    out=bass.AP(tensor=h8_in.tensor, offset=h8_in.offset,
                                ap=[[TC, 128], [128 * TC, KH], [1, TC]]),
                    in_=h8_stage)
                nc.gpsimd.collective_compute(
                    "AllGather", ALU.bypass, replica_groups=rg,
                    ins=[h8_in.opt()], outs=[h8_out.opt()])
                nc.gpsimd.collective_compute(
                    "AllGather", ALU.bypass, replica_groups=rg,
                    ins=[w_in.opt()], outs=[w_out.opt()])

            # ================= SHARED EXPERT (bf16, own chunk, in AG gap) ===
            with tc.tile_pool(name="shp", bufs=2) as shp, \
                 tc.tile_pool(name="wsp", bufs=2) as wsp, \
                 tc.tile_pool(name="ps_big", bufs=3, space="PSUM") as ps_big, \
                 tc.tile_pool(name="ps_sm", bufs=2, space="PSUM") as ps_sm:
                gus_bf = []
                for n in range(4):
                    wsg = wsp.tile([128, KH, 512], BF16, tag="wsg")
                    (nc.sync if n % 2 == 0 else nc.scalar).dma_start(
                        out=wsg,
                        in_=bass.AP(tensor=wsgu_l, offset=n * 128 * KH * 512,
                                    ap=[[KH * 512, 128], [1, KH * 512]]))
                    pgu = ps_big.tile([TC, 512], F32, tag="mm512")
                    for k in range(KH):
                        nc.tensor.matmul(pgu, h2Ts[k], wsg[:, k, :],
                                         start=(k == 0), stop=(k == KH - 1))
                    t_ = shp.tile([TC, 512], BF16, tag="gusbf", bufs=4,
                                  name=f"gus{n}")
                    if n < 2:
                        nc.scalar.activation(t_, pgu, AF.Silu)
                    else:
                        nc.scalar.copy(t_, pgu)
                    gus_bf.append(t_)
                actsT = []
                for n in range(2):
                    nc.vector.tensor_mul(gus_bf[n], gus_bf[n], gus_bf[n + 2])
                    for jj in range(4):
                        i = n * 4 + jj
                        pt = ps_sm.tile([128, 128], BF16, tag="pstb")
                        nc.tensor.transpose(
                            pt, gus_bf[n][:, jj * 128:(jj + 1) * 128], ident_bf)
                        t_ = shp.tile([128, TC], BF16, tag=f"actsT{i}", bufs=1,
                                      name=f"actsT{i}")
                        nc.vector.tensor_copy(t_, pt)
                        actsT.append(t_)
                for n in range(4):
                    wsd_sb = wsp.tile([128, 8, 512], BF16, tag="wsd")
                    (nc.sync if n % 2 == 0 else nc.scalar).dma_start(
                        out=wsd_sb,
                        in_=bass.AP(tensor=wsd_l, offset=n * 128 * 8 * 512,
                                    ap=[[8 * 512, 128], [1, 8 * 512]]))
                    psh = ps_big.tile([TC, 512], F32, tag="mm512")
                    for i in range(8):
                        nc.tensor.matmul(psh, actsT[i], wsd_sb[:, i, :],
                                         start=(i == 0), stop=(i == 7))
                    nc.vector.tensor_add(
                        shared_sb[:, n * 512:(n + 1) * 512], psh,
                        x2_sb[:, n * 512:(n + 1) * 512])

            # ================= ROUTED EXPERT (fp8 DoubleRow) ================
            with tc.tile_pool(name="moep", bufs=1) as moep, \
                 tc.tile_pool(name="gstage", bufs=4) as gstage, \
                 tc.tile_pool(name="ps_gu", bufs=4, space="PSUM") as ps_gu, \
                 tc.tile_pool(name="ps_dn", bufs=4, space="PSUM") as ps_dn:
                wgu8_sb = moep.tile([128, 16, 16, 2, 128], F8)
                nc.sync.dma_start(
                    out=wgu8_sb,
                    in_=bass.AP(tensor=wgu8_l, offset=0,
                                ap=[[16 * 16 * 2 * 128, 128],
                                    [1, 16 * 16 * 2 * 128]]))
                wd8_sb = moep.tile([128, 4, 2, 4, 512], F8)
                nc.scalar.dma_start(
                    out=wd8_sb,
                    in_=bass.AP(tensor=wd8_l, offset=0,
                                ap=[[4 * 2 * 4 * 512, 128], [1, 4 * 2 * 4 * 512]]))
                h2T8 = moep.tile([128, KH, T], F8)
                for k in range(KH):
                    (nc.sync if k % 2 == 0 else nc.scalar).dma_start(
                        out=h2T8[:, k, :],
                        in_=bass.AP(tensor=h8_out.tensor,
                                    offset=h8_out.offset + k * 128 * TC,
                                    ap=[[TC, 128], [H * TC, N_CORES], [1, TC]]))
                act8 = moep.tile([128, 8, T], F8)
                # own-expert weight column per chunk (esel carries the
                # 1/16 wd descale); emitted after the load issues so the
                # w_out reads don't head-of-line-block the expert loads.
                for t in range(T // TC):
                    wmt = sb.tile([TC, E], F32, tag="wmt")
                    nc.gpsimd.dma_start(out=wmt,
                                         in_=w_out[t * TC:(t + 1) * TC, :])
                    nc.vector.tensor_mul(wmt, wmt, esel_sb)
                    nc.vector.tensor_reduce(wcols[:, t:t + 1], wmt, axis=AX.X,
                                            op=ALU.add)

                # strip order (host layout): m even = g_{m//2}, m odd = u_{m//2}
                for nch in range(2):          # token chunk (512)
                    tsl = slice(nch * 512, (nch + 1) * 512)
                    for w in range(4):        # wave of 4 strips = 2 (g,u) pairs
                        pg = []
                        for s in range(4):
                            m = w * 4 + s
                            ps = ps_gu.tile([128, 512], F32, tag="psgu")
                            for kp in range(16):   # 8 hi + 8 lo residual pairs
                                nc.tensor.matmul(
                                    ps, wgu8_sb[:, m, kp, :, :],
                                    h2T8[:, 2 * (kp % 8):2 * (kp % 8) + 2, tsl],
                                    start=(kp == 0), stop=(kp == 15),
                                    perf_mode=DR)
                            pg.append(ps)
                        for pr_ in range(2):  # finish (g,u) pairs of this wave
                            i = w * 2 + pr_
                            g_bf = gstage.tile([128, 512], BF16, tag="gbf")
                            nc.scalar.activation(g_bf, pg[2 * pr_], AF.Silu,
                                                 scale=1.0 / WSCL)
                            u_bf = gstage.tile([128, 512], BF16, tag="ubf")
                            nc.scalar.mul(u_bf, pg[2 * pr_ + 1], 1.0 / WSCL)
                            nc.vector.tensor_mul(act8[:, i, tsl], g_bf, u_bf)
                    # down for this token chunk (4 t-blocks of 128)
                    for tb in range(4):
                        t = nch * 4 + tb
                        rt_sb = gstage.tile([TC, 8, 512], BF16, tag="rt",
                                            bufs=2)
                        for n in range(4):
                            pd = ps_dn.tile([TC, 512], F32, tag="psdn")
                            for kp in range(4):
                                nc.tensor.matmul(
                                    pd, act8[:, 2 * kp:2 * kp + 2,
                                             t * TC:(t + 1) * TC],
                                    wd8_sb[:, kp, :, n, :],
                                    start=(kp == 0), stop=(kp == 3),
                                    perf_mode=DR)
                            nc.vector.tensor_scalar_mul(
                                rt_sb[:, n, :], pd, wcols[:, t:t + 1])
                        nc.sync.dma_start(
                            out=rs_in[t * TC:(t + 1) * TC, :],
                            in_=rt_sb.rearrange("p n c -> p (n c)")
                            [:, 0:H])

                nc.gpsimd.collective_compute(
                    "ReduceScatter", ALU.add, replica_groups=rg,
                    ins=[rs_in.opt()], outs=[rs_out.opt()])

            moe_bf = sb.tile([TC, H], BF16, tag="moebf", bufs=1)
            nc.sync.dma_start(out=moe_bf, in_=rs_out[:, :])
            moe_f = sb.tile([TC, H], F32, tag="moef", bufs=1)
            nc.vector.tensor_copy(moe_f, moe_bf)
            nc.vector.tensor_add(moe_f, shared_sb, moe_f)
            nc.sync.dma_start(out=out_chunk[:, :], in_=moe_f)

    nc.compile()
    return nc


def _prep_inputs(hidden_states, w_ln1, w_ln2, wqkv, q_norm_w, k_norm_w, wo,
                 w_router, w_gu, w_d, ws_gu, ws_d, positions):
    import ml_dtypes
    bf = ml_dtypes.bfloat16
    f8 = ml_dtypes.float8_e4m3

    x = np.asarray(hidden_states, np.float32).reshape(T, H)
    w_ln1 = np.asarray(w_ln1, np.float32)
    w_ln2 = np.asarray(w_ln2, np.float32)
    wqkv_e = (np.asarray(wqkv, np.float32) * w_ln1[:, None]).astype(bf)
    wo_b = np.asarray(wo, np.float32).astype(bf)
    wgu_e = np.asarray(w_gu, np.float32) * w_ln2[None, :, None]
    wd_b = np.asarray(w_d, np.float32)
    wsgu_e = (np.asarray(ws_gu, np.float32) * w_ln2[:, None]).astype(bf)
    wsd_b = np.asarray(ws_d, np.float32).astype(bf)
    wrT_e = np.ascontiguousarray(
        (np.asarray(w_router, np.float32) * w_ln2[None, :]).T.astype(np.float32))

    # layouts: [chunk][partition][k*512+c] for streamed bf16 weights
    def chunks_pkc(wmat, nch, kh):
        # wmat [K, N] -> [nch, 128, kh*512]
        K, N = wmat.shape
        out = np.empty((nch, 128, kh * 512), wmat.dtype)
        for n in range(nch):
            blk = wmat[:, n * 512:(n + 1) * 512]           # [K, 512]
            out[n] = (blk.reshape(kh, 128, 512)
                      .transpose(1, 0, 2).reshape(128, kh * 512))
        return np.ascontiguousarray(out)

    wqkv_lh = chunks_pkc(wqkv_e, 6, KH)
    wo_lh = chunks_pkc(wo_b, 4, KH)
    wsgu_lh = chunks_pkc(wsgu_e, 4, KH)
    wsd_lh = chunks_pkc(wsd_b, 4, 8)

    pos = np.asarray(positions).astype(np.float64)
    inv_freq = 1.0 / (10000.0 ** (np.arange(0, DH, 2, dtype=np.float64) / DH))
    freqs = pos[:, None] * inv_freq[None, :]
    cos = np.cos(freqs).astype(np.float32)
    sin = np.sin(freqs).astype(np.float32)
    qw = np.asarray(q_norm_w, np.float32)
    kw = np.asarray(k_norm_w, np.float32)

    def rope_tab(w):
        return np.ascontiguousarray(
            np.stack([cos * w[None, :64], sin * w[None, 64:],
                      cos * w[None, 64:], sin * w[None, :64]], axis=1), np.float32)

    rq = rope_tab(qw)
    rk = rope_tab(kw)

    kidx = np.arange(T)
    in_maps = []
    for c in range(N_CORES):
        rows = np.arange(c * TC, (c + 1) * TC)
        mask = np.ascontiguousarray(
            np.where(rows[:, None] >= kidx[None, :], 0.0, NEG)
            .astype(np.float32).T)  # [T(tk), TC(tq)]
        es = np.zeros((1, E), np.float32)
        es[0, c] = 1.0 / WSCL  # folds the wd fp8 pre-scale back out

        # wgu8: [128 p][strip m 16][kp 8][pair 2][mcol 128]
        # strip m: even -> g cols (m//2)*128.., odd -> u cols I+(m//2)*128..
        wgs = wgu_e[c] * WSCL
        whi = wgs.astype(f8).astype(np.float32)
        wlo = (wgs - whi).astype(f8).astype(np.float32)    # residual, same scale
        wgu8 = np.empty((128, 16, 16, 2, 128), np.float32)
        for m in range(16):
            col0 = (m // 2) * 128 + (m % 2) * I
            for src_w, kp0 in ((whi, 0), (wlo, 8)):
                blk = src_w[:, col0:col0 + 128]            # [H, 128]
                wgu8[:, m, kp0:kp0 + 8] = (blk.reshape(8, 2, 128, 128)
                                           .transpose(2, 0, 1, 3))
        # wd8: [128 p][kp 4][pair 2][n 4][c 512]
        wde = (wd_b[c] * WSCL).astype(f8).astype(np.float32)
        wd8 = np.ascontiguousarray(
            wde.reshape(4, 2, 128, 4, 512).transpose(2, 0, 1, 3, 4))

        in_maps.append({
            "x_chunk": np.ascontiguousarray(x[c * TC:(c + 1) * TC]),
            "wqkv_l": wqkv_lh,
            "wo_l": wo_lh,
            "wgu8_l": wgu8.astype(f8),
            "wd8_l": wd8.astype(f8),
            "wsgu_l": wsgu_lh,
            "wsd_l": wsd_lh,
            "wrT": wrT_e,
            "rope_q": np.ascontiguousarray(rq[c * TC:(c + 1) * TC]),
            "rope_k": np.ascontiguousarray(rk[c * TC:(c + 1) * TC]),
            "mask_in": mask,
            "esel": es,
        })
    return in_maps


def kernel(**inputs):
    import os
    if "nc" not in _cache:
        _cache["nc"] = build_nc()
    nc = _cache["nc"]
    in_maps = _prep_inputs(**inputs)
    trace = bool(int(os.environ.get("KERNEL_TRACE", "0")))
    res = run_bass_kernel_spmd(nc, in_maps, core_ids=list(range(N_CORES)),
                               trace=trace)
    _cache["last_result"] = res
    out = np.concatenate(
        [res.results[c]["out_chunk"] for c in range(N_CORES)], axis=0)
    return out.reshape(1, T, H).astype(np.float32)


if __name__ == "__main__":
    import reference
    inp = {k: np.asarray(v) for k, v in reference.setup_inputs().items()}
    got = kernel(**inp)
    exp = np.asarray(reference.reference(**reference.setup_inputs()))
    denom = np.abs(exp).max()
    err = np.abs(got - exp).max() / denom
    print("abs max:", denom, "rel err:", err)
